# revision 16
# baseline (speedup 1.0000x reference)
"""CollisionLoss kernel for Trainium2 (8 NeuronCores, Bass/Tile).

Computes: sum over (future, box) of masked AABB-overlap area between the
ego box (per-future, from the sdc trajectory) and 1M gt boxes per future,
times WEIGHT.

Distribution (memory-bound problem):
 - future_gt_corners [6,1M,4,2] is sharded along the boxes axis across 8
   cores; each core emits 128 partial sums; host adds 8x128 in float64.
 - Host folds box_mask into the corner stream (masked box -> sentinel
   coords 15.0, whose clamped overlap is 0), quantizes the corners to
   fp8-e3m4 (validated rel err ~5e-4 vs the 2e-2 budget; |corner| <= 5.5
   fits e3m4's +-15.5 range), and deinterleaves each future's boxes into
   8 coordinate planes ordered [X0,Y0,X1,Y1 | X2,Y2,X3,Y3] so every tree
   op on the device is a single dense unit-stride tensor_tensor.
 - The ego AABB (24 scalars) is computed on host exactly as the
   reference does (O(1) work) and uploaded as per-partition scalars.

Per-core dataflow, per future chunk (w boxes/partition, 128 partitions):
  DMA (gpsimd/SWDGE): fp8 planes, [128, 8w]. SWDGE spreads across all 16
      SDMA engines (~190 GB/s/core measured) vs HWDGE's 5 (~112 GB/s).
  ACT: one fp8->fp16 upconvert (Copy) over the whole chunk.
  DVE L1 (2x mode): max/min of plane-halves -> (m1x,m1y,m2x,m2y) dense.
  DVE L2 (2x): combine -> (xb1,yb1), (xb2,yb2) dense.
  DVE clamp (4x): tensor_scalar vs per-partition ego scalars:
      hi = min(xb1,xa1)|min(yb1,ya1); lo = max(xb2,xa2)|max(yb2,ya2).
  DVE sub (2x): wh = hi - lo  (wr, hr interleaved by plane).
  ACT: hp = relu(hr).
  DVE area (1x STT): (wr max 0) * hp, fused per-partition f32 accumulate.
Chunks: future 0 split 4x (short pipeline head), future 5 split 2x
(short drain), middle futures whole.
"""

import numpy as np

DELTA = 0.5
WEIGHT = 1.0
W = 1.85 + DELTA
H = 4.084 + DELTA

F = 6
N = 1_000_000
CORES = 8
PER_CORE = N // CORES  # 125000
P = 128                # SBUF partitions
BPR = 980              # boxes per partition row (padded)
PADDED = P * BPR       # 125440 boxes per core
SENTINEL = 15.0        # masked/padding boxes -> zero overlap after clamp

# chunk widths per future (sum = BPR each)
CHUNKS = [
    [245, 245, 245, 245],
    [490, 490],
    [980],
    [980],
    [980],
    [490, 490],
]
NCHUNK = sum(len(c) for c in CHUNKS)

_prog = None
_prog_key = None
_last_in_maps = None


def _build_program(ego_vals):
    """ego_vals: [F][4] python floats (xa1, xa2, ya1, ya2) baked as immediates."""
    from contextlib import ExitStack

    import concourse.bacc as bacc
    import concourse.tile as tile
    from concourse import mybir

    Alu = mybir.AluOpType
    Act = mybir.ActivationFunctionType
    f8 = mybir.dt.float8e3
    f16 = mybir.dt.float16
    f32 = mybir.dt.float32

    nc = bacc.Bacc("TRN2", target_bir_lowering=False, debug=False)

    planes = [
        nc.dram_tensor(f"planes{f}", [P, 8 * BPR], f8, kind="ExternalInput")
        for f in range(F)
    ]
    PS = 512  # psum bank width (f32)
    out = nc.dram_tensor("out", [1, PS], f32, kind="ExternalOutput")

    # flat chunk list: (future, elem offset within future free dim, width)
    tiles = []
    for f in range(F):
        off = 0
        for w in CHUNKS[f]:
            tiles.append((f, off, w))
            off += 8 * w
    n_tiles = len(tiles)

    with tile.TileContext(nc) as tc, ExitStack() as ctx:
        const_pool = ctx.enter_context(tc.tile_pool(name="const", bufs=1))
        cpool = ctx.enter_context(tc.tile_pool(name="cd", bufs=3))
        upool = ctx.enter_context(tc.tile_pool(name="up", bufs=3))
        l1pool = ctx.enter_context(tc.tile_pool(name="l1", bufs=2))
        l2pool = ctx.enter_context(tc.tile_pool(name="l2", bufs=2))
        cspool = ctx.enter_context(tc.tile_pool(name="cs", bufs=2))
        spool = ctx.enter_context(tc.tile_pool(name="sm", bufs=3))

        psum_pool = ctx.enter_context(tc.tile_pool(name="ps", bufs=1, space="PSUM"))
        psum = psum_pool.tile([1, PS], f32)
        ones = const_pool.tile([P, 1], f16)
        nc.vector.memset(ones[:], 1.0)

        # Warm the ACT engine (pulls ACT_TABLE_LOAD into the DMA shadow so
        # the first real upconvert doesn't pay it).
        warm = const_pool.tile([P, 8], f16)
        nc.vector.memset(warm[:], 0.0)
        nc.scalar.activation(out=warm[:], in_=warm[:], func=Act.Relu)

        state = {}
        mm_state = {"n": 0, "total": n_tiles + sum(1 for f in range(F) for w in CHUNKS[f] if w > PS)}

        def s0_dma(t):
            f, off, w = tiles[t]
            st = state[t] = {}
            cd = cpool.tile([P, 8 * w], f8, tag="cd")
            # First chunks ride the HWDGE queues (their engines come out of
            # preamble ~3.5us before gpsimd's SWDGE path); steady state uses
            # SWDGE which spreads over all 16 SDMA engines.
            eng = {0: nc.sync, 1: nc.scalar, 2: nc.sync, 3: nc.scalar}.get(t, nc.gpsimd)
            eng.dma_start(out=cd[:], in_=planes[f].ap()[:, off : off + 8 * w])
            st["cd"] = cd

        def s1_up(t):
            if t == 0:
                return  # chunk 0's L1 reads fp8 directly (fast pipeline start)
            f, off, w = tiles[t]
            st = state[t]
            u = upool.tile([P, 8 * w], f16, tag="u")
            nc.scalar.activation(out=u[:], in_=st["cd"][:], func=Act.Copy)
            st["u"] = u

        def s2_l1(t):
            f, off, w = tiles[t]
            st = state[t]
            u = st["cd"] if t == 0 else st["u"]
            mx = l1pool.tile([P, 4 * w], f16, tag="mx")
            mn = l1pool.tile([P, 4 * w], f16, tag="mn")
            nc.vector.tensor_tensor(
                out=mx[:], in0=u[:, 0 : 4 * w], in1=u[:, 4 * w : 8 * w], op=Alu.max
            )
            nc.vector.tensor_tensor(
                out=mn[:], in0=u[:, 0 : 4 * w], in1=u[:, 4 * w : 8 * w], op=Alu.min
            )
            st["mx"], st["mn"] = mx, mn

        def s3_l2(t):
            f, off, w = tiles[t]
            st = state[t]
            mx, mn = st["mx"], st["mn"]
            bx = l2pool.tile([P, 2 * w], f16, tag="bx")  # (xb1, yb1)
            bn = l2pool.tile([P, 2 * w], f16, tag="bn")  # (xb2, yb2)
            nc.vector.tensor_tensor(
                out=bx[:], in0=mx[:, 0 : 2 * w], in1=mx[:, 2 * w : 4 * w], op=Alu.max
            )
            nc.vector.tensor_tensor(
                out=bn[:], in0=mn[:, 0 : 2 * w], in1=mn[:, 2 * w : 4 * w], op=Alu.min
            )
            st["bx"], st["bn"] = bx, bn

        def s4_cs(t):
            f, off, w = tiles[t]
            st = state[t]
            bx, bn = st["bx"], st["bn"]
            xa1, xa2, ya1, ya2 = ego_vals[f]
            hi = cspool.tile([P, 2 * w], f16, tag="hi")
            lo = cspool.tile([P, 2 * w], f16, tag="lo")
            nc.vector.tensor_scalar(
                out=hi[:, 0:w], in0=bx[:, 0:w], scalar1=xa1, scalar2=None, op0=Alu.min
            )
            nc.vector.tensor_scalar(
                out=hi[:, w : 2 * w], in0=bx[:, w : 2 * w], scalar1=ya1, scalar2=None,
                op0=Alu.min,
            )
            nc.vector.tensor_scalar(
                out=lo[:, 0:w], in0=bn[:, 0:w], scalar1=xa2, scalar2=None, op0=Alu.max
            )
            nc.vector.tensor_scalar(
                out=lo[:, w : 2 * w], in0=bn[:, w : 2 * w], scalar1=ya2, scalar2=None,
                op0=Alu.max,
            )
            wh = cspool.tile([P, 2 * w], f16, tag="wh")
            nc.vector.tensor_tensor(out=wh[:], in0=hi[:], in1=lo[:], op=Alu.subtract)
            st["wh"] = wh

        def s5_relu(t):
            f, off, w = tiles[t]
            st = state[t]
            whp = spool.tile([P, 2 * w], f16, tag="whp")
            nc.scalar.activation(out=whp[:], in_=st["wh"][:], func=Act.Relu)
            st["whp"] = whp

        def s6_area(t):
            f, off, w = tiles[t]
            st = state[t]
            whp = st["whp"]
            terms = spool.tile([P, w], f16, tag="terms")
            nc.vector.tensor_tensor(
                out=terms[:], in0=whp[:, 0:w], in1=whp[:, w : 2 * w], op=Alu.mult
            )
            # PE: sum across partitions into psum[0, 0:chunkw], accumulated
            # over all chunks (overlapping ranges add).
            for a in range(0, w, PS):
                b = min(w, a + PS)
                mm_state["n"] += 1
                nc.tensor.matmul(
                    out=psum[0:1, 0 : b - a],
                    lhsT=ones[:],
                    rhs=terms[:, a:b],
                    start=(mm_state["n"] == 1),
                    stop=(mm_state["n"] == mm_state["total"]),
                )
            del state[t]

        stages = [s0_dma, s1_up, s2_l1, s3_l2, s4_cs, s5_relu, s6_area]
        for t in range(n_tiles + len(stages) - 1):
            for k, fn in enumerate(stages):
                tt = t - k
                if 0 <= tt < n_tiles:
                    fn(tt)

        # psum -> SBUF -> HBM; host does the final 512-wide reduction.
        pout = const_pool.tile([1, PS], f32)
        nc.vector.tensor_copy(pout[:], psum[:])
        nc.sync.dma_start(out=out.ap(), in_=pout[:])

    nc.compile()
    return nc


def _get_prog(ego_vals):
    global _prog, _prog_key
    key = tuple(tuple(r) for r in ego_vals)
    if _prog is None or _prog_key != key:
        _prog = _build_program(ego_vals)
        _prog_key = key
    return _prog


def _ego_aabb(sdc_traj_all, sdc_planning_gt):
    """Per-future ego AABB [F,4] = (xa1, xa2, ya1, ya2), mirroring reference."""
    sdc_traj_all = np.asarray(sdc_traj_all, dtype=np.float32)
    sdc_planning_gt = np.asarray(sdc_planning_gt, dtype=np.float32)
    x = sdc_traj_all[0, :, 0]
    y = sdc_traj_all[0, :, 1]
    theta = sdc_planning_gt[0, :, 2]
    local = np.array(
        [[W / 2, -H / 2], [W / 2, H / 2], [-W / 2, H / 2], [-W / 2, -H / 2]],
        dtype=np.float32,
    )
    c, s = np.cos(theta), np.sin(theta)
    rot = np.stack([np.stack([c, s], -1), np.stack([-s, c], -1)], -2)  # [F,2,2]
    corners = np.einsum("fij,kj->fki", rot, local) + np.stack([x, y], -1)[:, None, :]
    corners = corners.astype(np.float32)
    xa1 = corners[..., 0].max(-1)
    ya1 = corners[..., 1].max(-1)
    xa2 = corners[..., 0].min(-1)
    ya2 = corners[..., 1].min(-1)
    return np.stack([xa1, xa2, ya1, ya2], -1).astype(np.float32)  # [F,4]


def _layout_core(q8core):
    """[F, PER_CORE, 4, 2] fp8 -> {planes_f: [P, 8*BPR]} in chunked order."""
    import ml_dtypes

    pad = np.full((F, PADDED - PER_CORE, 4, 2), SENTINEL, dtype=ml_dtypes.float8_e3m4)
    a = np.concatenate([q8core, pad], axis=1)  # [F, PADDED, 4, 2]
    # [F, P, BPR, 4, 2] -> planes [F, P, 8, BPR], plane idx q = corner*2+coord
    a = a.reshape(F, P, BPR, 8).transpose(0, 1, 3, 2)
    outs = {}
    for f in range(F):
        blocks = []
        j = 0
        for w in CHUNKS[f]:
            blocks.append(a[f, :, :, j : j + w].reshape(P, 8 * w))
            j += w
        outs[f"planes{f}"] = np.ascontiguousarray(np.concatenate(blocks, axis=1))
    return outs


def kernel(sdc_traj_all, sdc_planning_gt, sdc_planning_gt_mask, future_gt_corners, box_mask):
    import ml_dtypes
    from concourse.bass_utils import run_bass_kernel_spmd

    corners = np.asarray(future_gt_corners, dtype=np.float32)
    mask = np.asarray(box_mask)
    masked = np.where(mask[..., None, None] != 0, corners, np.float32(SENTINEL))
    q8 = masked.astype(ml_dtypes.float8_e3m4)  # [F, N, 4, 2]

    eg = _ego_aabb(sdc_traj_all, sdc_planning_gt)  # [F,4] = (xa1, xa2, ya1, ya2)
    ego_vals = [[float(eg[f, k]) for k in range(4)] for f in range(F)]

    in_maps = []
    for cidx in range(CORES):
        lo, hi = cidx * PER_CORE, (cidx + 1) * PER_CORE
        in_maps.append(_layout_core(q8[:, lo:hi]))

    global _last_in_maps
    _last_in_maps = in_maps
    res = run_bass_kernel_spmd(_get_prog(ego_vals), in_maps, list(range(CORES))).results
    total = 0.0
    for r in res:
        total += float(r["out"].astype(np.float64).sum())
    return np.array([total], dtype=np.float32) * np.float32(WEIGHT)


# revision 18
# speedup vs baseline: 1.0361x; 1.0361x over previous
"""CollisionLoss kernel for Trainium2 (8 NeuronCores, Bass/Tile).

Computes: sum over (future, box) of masked AABB-overlap area between the
ego box (per-future, from the sdc trajectory) and 1M gt boxes per future,
times WEIGHT.

Distribution (memory-bound problem):
 - future_gt_corners [6,1M,4,2] is sharded along the boxes axis across 8
   cores; each core emits 128 partial sums; host adds 8x128 in float64.
 - Host folds box_mask into the corner stream (masked box -> sentinel
   coords 15.0, whose clamped overlap is 0), quantizes the corners to
   fp8-e3m4 (validated rel err ~5e-4 vs the 2e-2 budget; |corner| <= 5.5
   fits e3m4's +-15.5 range), and deinterleaves each future's boxes into
   8 coordinate planes ordered [X0,Y0,X1,Y1 | X2,Y2,X3,Y3] so every tree
   op on the device is a single dense unit-stride tensor_tensor.
 - The ego AABB (24 scalars) is computed on host exactly as the
   reference does (O(1) work) and uploaded as per-partition scalars.

Per-core dataflow, per future chunk (w boxes/partition, 128 partitions):
  DMA (gpsimd/SWDGE): fp8 planes, [128, 8w]. SWDGE spreads across all 16
      SDMA engines (~190 GB/s/core measured) vs HWDGE's 5 (~112 GB/s).
  ACT: one fp8->fp16 upconvert (Copy) over the whole chunk.
  DVE L1 (2x mode): max/min of plane-halves -> (m1x,m1y,m2x,m2y) dense.
  DVE L2 (2x): combine -> (xb1,yb1), (xb2,yb2) dense.
  DVE clamp (4x): tensor_scalar vs per-partition ego scalars:
      hi = min(xb1,xa1)|min(yb1,ya1); lo = max(xb2,xa2)|max(yb2,ya2).
  DVE sub (2x): wh = hi - lo  (wr, hr interleaved by plane).
  ACT: hp = relu(hr).
  DVE area (1x STT): (wr max 0) * hp, fused per-partition f32 accumulate.
Chunks: future 0 split 4x (short pipeline head), future 5 split 2x
(short drain), middle futures whole.
"""

import numpy as np

DELTA = 0.5
WEIGHT = 1.0
W = 1.85 + DELTA
H = 4.084 + DELTA

F = 6
N = 1_000_000
CORES = 8
PER_CORE = N // CORES  # 125000
P = 128                # SBUF partitions
BPR = 980              # boxes per partition row (padded)
PADDED = P * BPR       # 125440 boxes per core
SENTINEL = 15.0        # masked/padding boxes -> zero overlap after clamp

# chunk widths per future (sum = BPR each)
CHUNKS = [
    [245, 245, 245, 245],
    [490, 490],
    [490, 490],
    [980],
    [980],
    [490, 490],
]
NCHUNK = sum(len(c) for c in CHUNKS)

_prog = None
_prog_key = None
_last_in_maps = None


def _build_program(ego_vals):
    """ego_vals: [F][4] python floats (xa1, xa2, ya1, ya2) baked as immediates."""
    from contextlib import ExitStack

    import concourse.bacc as bacc
    import concourse.tile as tile
    from concourse import mybir

    Alu = mybir.AluOpType
    Act = mybir.ActivationFunctionType
    f8 = mybir.dt.float8e3
    f16 = mybir.dt.float16
    f32 = mybir.dt.float32

    nc = bacc.Bacc("TRN2", target_bir_lowering=False, debug=False)

    planes = [
        nc.dram_tensor(f"planes{f}", [P, 8 * BPR], f8, kind="ExternalInput")
        for f in range(F)
    ]
    PS = 512  # psum bank width (f32)
    out = nc.dram_tensor("out", [1, PS], f32, kind="ExternalOutput")

    # flat chunk list: (future, elem offset within future free dim, width)
    tiles = []
    for f in range(F):
        off = 0
        for w in CHUNKS[f]:
            tiles.append((f, off, w))
            off += 8 * w
    n_tiles = len(tiles)

    with tile.TileContext(nc) as tc, ExitStack() as ctx:
        const_pool = ctx.enter_context(tc.tile_pool(name="const", bufs=1))
        cpool = ctx.enter_context(tc.tile_pool(name="cd", bufs=3))
        upool = ctx.enter_context(tc.tile_pool(name="up", bufs=3))
        l1pool = ctx.enter_context(tc.tile_pool(name="l1", bufs=2))
        l2pool = ctx.enter_context(tc.tile_pool(name="l2", bufs=2))
        cspool = ctx.enter_context(tc.tile_pool(name="cs", bufs=2))
        spool = ctx.enter_context(tc.tile_pool(name="sm", bufs=3))

        psum_pool = ctx.enter_context(tc.tile_pool(name="ps", bufs=1, space="PSUM"))
        psum = psum_pool.tile([1, PS], f32)
        ones = const_pool.tile([P, 1], f16)
        nc.vector.memset(ones[:], 1.0)

        # Warm the ACT engine (pulls ACT_TABLE_LOAD into the DMA shadow so
        # the first real upconvert doesn't pay it).
        warm = const_pool.tile([P, 8], f16)
        nc.vector.memset(warm[:], 0.0)
        nc.scalar.activation(out=warm[:], in_=warm[:], func=Act.Relu)

        state = {}
        mm_state = {"n": 0, "total": n_tiles + sum(1 for f in range(F) for w in CHUNKS[f] if w > PS)}

        def s0_dma(t):
            f, off, w = tiles[t]
            st = state[t] = {}
            cd = cpool.tile([P, 8 * w], f8, tag="cd")
            nc.gpsimd.dma_start(out=cd[:], in_=planes[f].ap()[:, off : off + 8 * w])
            st["cd"] = cd

        def s1_up(t):
            if t == 0:
                return  # chunk 0's L1 reads fp8 directly (fast pipeline start)
            f, off, w = tiles[t]
            st = state[t]
            u = upool.tile([P, 8 * w], f16, tag="u")
            nc.scalar.activation(out=u[:], in_=st["cd"][:], func=Act.Copy)
            st["u"] = u

        def s2_l1(t):
            f, off, w = tiles[t]
            st = state[t]
            u = st["cd"] if t == 0 else st["u"]
            mx = l1pool.tile([P, 4 * w], f16, tag="mx")
            mn = l1pool.tile([P, 4 * w], f16, tag="mn")
            nc.vector.tensor_tensor(
                out=mx[:], in0=u[:, 0 : 4 * w], in1=u[:, 4 * w : 8 * w], op=Alu.max
            )
            nc.vector.tensor_tensor(
                out=mn[:], in0=u[:, 0 : 4 * w], in1=u[:, 4 * w : 8 * w], op=Alu.min
            )
            st["mx"], st["mn"] = mx, mn

        def s3_l2(t):
            f, off, w = tiles[t]
            st = state[t]
            mx, mn = st["mx"], st["mn"]
            bx = l2pool.tile([P, 2 * w], f16, tag="bx")  # (xb1, yb1)
            bn = l2pool.tile([P, 2 * w], f16, tag="bn")  # (xb2, yb2)
            nc.vector.tensor_tensor(
                out=bx[:], in0=mx[:, 0 : 2 * w], in1=mx[:, 2 * w : 4 * w], op=Alu.max
            )
            nc.vector.tensor_tensor(
                out=bn[:], in0=mn[:, 0 : 2 * w], in1=mn[:, 2 * w : 4 * w], op=Alu.min
            )
            st["bx"], st["bn"] = bx, bn

        def s4_cs(t):
            f, off, w = tiles[t]
            st = state[t]
            bx, bn = st["bx"], st["bn"]
            xa1, xa2, ya1, ya2 = ego_vals[f]
            hi = cspool.tile([P, 2 * w], f16, tag="hi")
            lo = cspool.tile([P, 2 * w], f16, tag="lo")
            nc.vector.tensor_scalar(
                out=hi[:, 0:w], in0=bx[:, 0:w], scalar1=xa1, scalar2=None, op0=Alu.min
            )
            nc.vector.tensor_scalar(
                out=hi[:, w : 2 * w], in0=bx[:, w : 2 * w], scalar1=ya1, scalar2=None,
                op0=Alu.min,
            )
            nc.vector.tensor_scalar(
                out=lo[:, 0:w], in0=bn[:, 0:w], scalar1=xa2, scalar2=None, op0=Alu.max
            )
            nc.vector.tensor_scalar(
                out=lo[:, w : 2 * w], in0=bn[:, w : 2 * w], scalar1=ya2, scalar2=None,
                op0=Alu.max,
            )
            wh = cspool.tile([P, 2 * w], f16, tag="wh")
            nc.vector.tensor_tensor(out=wh[:], in0=hi[:], in1=lo[:], op=Alu.subtract)
            st["wh"] = wh

        def s5_relu(t):
            f, off, w = tiles[t]
            st = state[t]
            whp = spool.tile([P, 2 * w], f16, tag="whp")
            nc.scalar.activation(out=whp[:], in_=st["wh"][:], func=Act.Relu)
            st["whp"] = whp

        def s6_area(t):
            f, off, w = tiles[t]
            st = state[t]
            whp = st["whp"]
            terms = spool.tile([P, w], f16, tag="terms")
            nc.vector.tensor_tensor(
                out=terms[:], in0=whp[:, 0:w], in1=whp[:, w : 2 * w], op=Alu.mult
            )
            # PE: sum across partitions into psum[0, 0:chunkw], accumulated
            # over all chunks (overlapping ranges add).
            for a in range(0, w, PS):
                b = min(w, a + PS)
                mm_state["n"] += 1
                nc.tensor.matmul(
                    out=psum[0:1, 0 : b - a],
                    lhsT=ones[:],
                    rhs=terms[:, a:b],
                    start=(mm_state["n"] == 1),
                    stop=(mm_state["n"] == mm_state["total"]),
                )
            del state[t]

        stages = [s0_dma, s1_up, s2_l1, s3_l2, s4_cs, s5_relu, s6_area]
        for t in range(n_tiles + len(stages) - 1):
            for k, fn in enumerate(stages):
                tt = t - k
                if 0 <= tt < n_tiles:
                    fn(tt)

        # psum -> SBUF -> HBM; host does the final 512-wide reduction.
        pout = const_pool.tile([1, PS], f32)
        nc.vector.tensor_copy(pout[:], psum[:])
        nc.sync.dma_start(out=out.ap(), in_=pout[:])

    nc.compile()
    return nc


def _get_prog(ego_vals):
    global _prog, _prog_key
    key = tuple(tuple(r) for r in ego_vals)
    if _prog is None or _prog_key != key:
        _prog = _build_program(ego_vals)
        _prog_key = key
    return _prog


def _ego_aabb(sdc_traj_all, sdc_planning_gt):
    """Per-future ego AABB [F,4] = (xa1, xa2, ya1, ya2), mirroring reference."""
    sdc_traj_all = np.asarray(sdc_traj_all, dtype=np.float32)
    sdc_planning_gt = np.asarray(sdc_planning_gt, dtype=np.float32)
    x = sdc_traj_all[0, :, 0]
    y = sdc_traj_all[0, :, 1]
    theta = sdc_planning_gt[0, :, 2]
    local = np.array(
        [[W / 2, -H / 2], [W / 2, H / 2], [-W / 2, H / 2], [-W / 2, -H / 2]],
        dtype=np.float32,
    )
    c, s = np.cos(theta), np.sin(theta)
    rot = np.stack([np.stack([c, s], -1), np.stack([-s, c], -1)], -2)  # [F,2,2]
    corners = np.einsum("fij,kj->fki", rot, local) + np.stack([x, y], -1)[:, None, :]
    corners = corners.astype(np.float32)
    xa1 = corners[..., 0].max(-1)
    ya1 = corners[..., 1].max(-1)
    xa2 = corners[..., 0].min(-1)
    ya2 = corners[..., 1].min(-1)
    return np.stack([xa1, xa2, ya1, ya2], -1).astype(np.float32)  # [F,4]


def _layout_core(q8core):
    """[F, PER_CORE, 4, 2] fp8 -> {planes_f: [P, 8*BPR]} in chunked order."""
    import ml_dtypes

    pad = np.full((F, PADDED - PER_CORE, 4, 2), SENTINEL, dtype=ml_dtypes.float8_e3m4)
    a = np.concatenate([q8core, pad], axis=1)  # [F, PADDED, 4, 2]
    # [F, P, BPR, 4, 2] -> planes [F, P, 8, BPR], plane idx q = corner*2+coord
    a = a.reshape(F, P, BPR, 8).transpose(0, 1, 3, 2)
    outs = {}
    for f in range(F):
        blocks = []
        j = 0
        for w in CHUNKS[f]:
            blocks.append(a[f, :, :, j : j + w].reshape(P, 8 * w))
            j += w
        outs[f"planes{f}"] = np.ascontiguousarray(np.concatenate(blocks, axis=1))
    return outs


def kernel(sdc_traj_all, sdc_planning_gt, sdc_planning_gt_mask, future_gt_corners, box_mask):
    import ml_dtypes
    from concourse.bass_utils import run_bass_kernel_spmd

    corners = np.asarray(future_gt_corners, dtype=np.float32)
    mask = np.asarray(box_mask)
    masked = np.where(mask[..., None, None] != 0, corners, np.float32(SENTINEL))
    q8 = masked.astype(ml_dtypes.float8_e3m4)  # [F, N, 4, 2]

    eg = _ego_aabb(sdc_traj_all, sdc_planning_gt)  # [F,4] = (xa1, xa2, ya1, ya2)
    ego_vals = [[float(eg[f, k]) for k in range(4)] for f in range(F)]

    in_maps = []
    for cidx in range(CORES):
        lo, hi = cidx * PER_CORE, (cidx + 1) * PER_CORE
        in_maps.append(_layout_core(q8[:, lo:hi]))

    global _last_in_maps
    _last_in_maps = in_maps
    res = run_bass_kernel_spmd(_get_prog(ego_vals), in_maps, list(range(CORES))).results
    total = 0.0
    for r in res:
        total += float(r["out"].astype(np.float64).sum())
    return np.array([total], dtype=np.float32) * np.float32(WEIGHT)


# revision 26
# speedup vs baseline: 1.0801x; 1.0425x over previous
"""CollisionLoss kernel for Trainium2 (8 NeuronCores, Bass/Tile).

Computes: sum over (future, box) of masked AABB-overlap area between the
ego box (per-future, from the sdc trajectory) and 1M gt boxes per future,
times WEIGHT.

Distribution (memory-bound problem):
 - future_gt_corners [6,1M,4,2] is sharded along the boxes axis across 8
   cores; each core emits 128 partial sums; host adds 8x128 in float64.
 - Host folds box_mask into the corner stream (masked box -> sentinel
   coords 15.0, whose clamped overlap is 0), quantizes the corners to
   fp8-e3m4 (validated rel err ~5e-4 vs the 2e-2 budget; |corner| <= 5.5
   fits e3m4's +-15.5 range), and deinterleaves each future's boxes into
   8 coordinate planes ordered [X0,Y0,X1,Y1 | X2,Y2,X3,Y3] so every tree
   op on the device is a single dense unit-stride tensor_tensor.
 - The ego AABB (24 scalars) is computed on host exactly as the
   reference does (O(1) work) and uploaded as per-partition scalars.

Per-core dataflow, per future chunk (w boxes/partition, 128 partitions):
  DMA (gpsimd/SWDGE): fp8 planes, [128, 8w]. SWDGE spreads across all 16
      SDMA engines (~190 GB/s/core measured) vs HWDGE's 5 (~112 GB/s).
  ACT: one fp8->fp16 upconvert (Copy) over the whole chunk.
  DVE L1 (2x mode): max/min of plane-halves -> (m1x,m1y,m2x,m2y) dense.
  DVE L2 (2x): combine -> (xb1,yb1), (xb2,yb2) dense.
  DVE clamp (4x): tensor_scalar vs per-partition ego scalars:
      hi = min(xb1,xa1)|min(yb1,ya1); lo = max(xb2,xa2)|max(yb2,ya2).
  DVE sub (2x): wh = hi - lo  (wr, hr interleaved by plane).
  ACT: hp = relu(hr).
  DVE area (1x STT): (wr max 0) * hp, fused per-partition f32 accumulate.
Chunks: future 0 split 4x (short pipeline head), future 5 split 2x
(short drain), middle futures whole.
"""

import numpy as np

DELTA = 0.5
WEIGHT = 1.0
W = 1.85 + DELTA
H = 4.084 + DELTA

F = 6
N = 1_000_000
CORES = 8
PER_CORE = N // CORES  # 125000
P = 128                # SBUF partitions
BPR = 980              # boxes per partition row (padded)
PADDED = P * BPR       # 125440 boxes per core
SENTINEL = 15.0        # masked/padding boxes -> zero overlap after clamp

# chunk widths per future (sum = BPR each)
CHUNKS = [
    [245, 245, 245, 245],
    [490, 490],
    [490, 490],
    [980],
    [980],
    [490, 490],
]
NCHUNK = sum(len(c) for c in CHUNKS)

_prog = None
_prog_key = None
_last_in_maps = None


def _build_program(ego_vals):
    """ego_vals: [F][4] python floats (xa1, xa2, ya1, ya2) baked as immediates."""
    from contextlib import ExitStack

    import concourse.bacc as bacc
    import concourse.tile as tile
    from concourse import mybir

    Alu = mybir.AluOpType
    Act = mybir.ActivationFunctionType
    f8 = mybir.dt.float8e3
    f16 = mybir.dt.float16
    f32 = mybir.dt.float32

    nc = bacc.Bacc("TRN2", target_bir_lowering=False, debug=False)

    pl16 = [
        nc.dram_tensor(f"pl16_{f}", [P, 4 * BPR], f16, kind="ExternalInput")
        for f in range(F)
    ]
    pl8 = [
        nc.dram_tensor(f"pl8_{f}", [P, 4 * BPR], f8, kind="ExternalInput")
        for f in range(F)
    ]
    PS = 512  # psum bank width (f32)
    out = nc.dram_tensor("out", [1, PS], f32, kind="ExternalOutput")

    # flat chunk list: (future, elem offset within future free dim, width)
    tiles = []
    for f in range(F):
        off = 0
        for w in CHUNKS[f]:
            tiles.append((f, off, w))
            off += 8 * w
    n_tiles = len(tiles)

    with tile.TileContext(nc) as tc, ExitStack() as ctx:
        const_pool = ctx.enter_context(tc.tile_pool(name="const", bufs=1))
        cpool = ctx.enter_context(tc.tile_pool(name="cd", bufs=3))
        upool = ctx.enter_context(tc.tile_pool(name="up", bufs=3))
        l1pool = ctx.enter_context(tc.tile_pool(name="l1", bufs=2))
        l2pool = ctx.enter_context(tc.tile_pool(name="l2", bufs=2))
        cspool = ctx.enter_context(tc.tile_pool(name="cs", bufs=2))
        spool = ctx.enter_context(tc.tile_pool(name="sm", bufs=3))

        psum_pool = ctx.enter_context(tc.tile_pool(name="ps", bufs=1, space="PSUM"))
        psum = psum_pool.tile([1, PS], f32)
        ones = const_pool.tile([P, 1], f16)
        nc.vector.memset(ones[:], 1.0)

        # ACT relu biases per future: [xa1, ya1, -xa2, -ya2] (memset in the
        # preamble shadow; gpsimd so the vector queue stays clear).
        ego_b = const_pool.tile([P, 4 * F], f32)
        for f in range(F):
            xa1, xa2, ya1, ya2 = ego_vals[f]
            for k, v in enumerate([xa1, ya1, -xa2, -ya2]):
                nc.gpsimd.memset(ego_b[:, 4 * f + k : 4 * f + k + 1], float(v))

        # Warm the ACT engine (pulls ACT_TABLE_LOAD into the DMA shadow so
        # the first real upconvert doesn't pay it).
        warm = const_pool.tile([P, 8], f16)
        nc.vector.memset(warm[:], 0.0)
        nc.scalar.activation(out=warm[:], in_=warm[:], func=Act.Relu)

        state = {}
        mm_state = {"n": 0, "total": n_tiles + sum(1 for f in range(F) for w in CHUNKS[f] if w > PS)}

        def s0_dma(t):
            f, off, w = tiles[t]
            st = state[t] = {}
            hoff = off // 2  # offset within the 4-plane tensors
            cd16 = cpool.tile([P, 4 * w], f16, tag="cd16")
            cd8 = cpool.tile([P, 4 * w], f8, tag="cd8")
            nc.gpsimd.dma_start(out=cd16[:], in_=pl16[f].ap()[:, hoff : hoff + 4 * w])
            nc.gpsimd.dma_start(out=cd8[:], in_=pl8[f].ap()[:, hoff : hoff + 4 * w])
            st["cd16"], st["cd8"] = cd16, cd8

        def s1_up(t):
            if t == 0:
                return  # chunk 0's L1 reads fp8 directly (fast pipeline start)
            f, off, w = tiles[t]
            st = state[t]
            u = upool.tile([P, 4 * w], f16, tag="u")
            nc.scalar.activation(out=u[:], in_=st["cd8"][:], func=Act.Copy)
            st["u"] = u

        def s2_l1(t):
            f, off, w = tiles[t]
            st = state[t]
            u = st["cd8"] if t == 0 else st["u"]
            mx = l1pool.tile([P, 4 * w], f16, tag="mx")
            mn = l1pool.tile([P, 4 * w], f16, tag="mn")
            nc.vector.tensor_tensor(
                out=mx[:], in0=st["cd16"][:], in1=u[:], op=Alu.max
            )
            nc.vector.tensor_tensor(
                out=mn[:], in0=st["cd16"][:], in1=u[:], op=Alu.min
            )
            st["mx"], st["mn"] = mx, mn

        def s3_l2(t):
            f, off, w = tiles[t]
            st = state[t]
            mx, mn = st["mx"], st["mn"]
            bx = l2pool.tile([P, 2 * w], f16, tag="bx")  # (xb1, yb1)
            bn = l2pool.tile([P, 2 * w], f16, tag="bn")  # (xb2, yb2)
            nc.vector.tensor_tensor(
                out=bx[:], in0=mx[:, 0 : 2 * w], in1=mx[:, 2 * w : 4 * w], op=Alu.max
            )
            nc.vector.tensor_tensor(
                out=bn[:], in0=mn[:, 0 : 2 * w], in1=mn[:, 2 * w : 4 * w], op=Alu.min
            )
            st["bx"], st["bn"] = bx, bn

        def s4_r(t):
            # ACT: r1 = relu(ego_hi - b_hi), r2 = relu(b_lo - ego_lo).
            # Then w_pos = relu(W_f - r1x - r2x) etc (scalar folds below).
            f, off, w = tiles[t]
            st = state[t]
            bx, bn = st["bx"], st["bn"]
            xa1, xa2, ya1, ya2 = ego_vals[f]
            r1 = cspool.tile([P, 2 * w], f16, tag="r1")
            r2 = cspool.tile([P, 2 * w], f16, tag="r2")
            nc.scalar.activation(
                out=r1[:, 0:w], in_=bx[:, 0:w], func=Act.Relu, scale=-1.0,
                bias=ego_b[:, 4 * f + 0 : 4 * f + 1],
            )
            nc.scalar.activation(
                out=r1[:, w : 2 * w], in_=bx[:, w : 2 * w], func=Act.Relu, scale=-1.0,
                bias=ego_b[:, 4 * f + 1 : 4 * f + 2],
            )
            nc.scalar.activation(
                out=r2[:, 0:w], in_=bn[:, 0:w], func=Act.Relu,
                bias=ego_b[:, 4 * f + 2 : 4 * f + 3],
            )
            nc.scalar.activation(
                out=r2[:, w : 2 * w], in_=bn[:, w : 2 * w], func=Act.Relu,
                bias=ego_b[:, 4 * f + 3 : 4 * f + 4],
            )
            st["r1"], st["r2"] = r1, r2

        def s5_wneg(t):
            # DVE: t = r1 + r2; wn = min(t - W_f, 0) = -relu(w), per coord.
            f, off, w = tiles[t]
            st = state[t]
            xa1, xa2, ya1, ya2 = ego_vals[f]
            tt = cspool.tile([P, 2 * w], f16, tag="tt")
            nc.vector.tensor_tensor(out=tt[:], in0=st["r1"][:], in1=st["r2"][:], op=Alu.add)
            wn = spool.tile([P, 2 * w], f16, tag="wn")
            nc.vector.tensor_scalar(
                out=wn[:, 0:w], in0=tt[:, 0:w], scalar1=xa1 - xa2, scalar2=0.0,
                op0=Alu.subtract, op1=Alu.min,
            )
            nc.vector.tensor_scalar(
                out=wn[:, w : 2 * w], in0=tt[:, w : 2 * w], scalar1=ya1 - ya2,
                scalar2=0.0, op0=Alu.subtract, op1=Alu.min,
            )
            st["wn"] = wn

        def s6_area(t):
            f, off, w = tiles[t]
            st = state[t]
            wn = st["wn"]
            terms = spool.tile([P, w], f16, tag="terms")
            nc.vector.tensor_tensor(
                out=terms[:], in0=wn[:, 0:w], in1=wn[:, w : 2 * w], op=Alu.mult
            )
            # PE: sum across partitions into psum[0, 0:chunkw], accumulated
            # over all chunks (overlapping ranges add).
            for a in range(0, w, PS):
                b = min(w, a + PS)
                mm_state["n"] += 1
                nc.tensor.matmul(
                    out=psum[0:1, 0 : b - a],
                    lhsT=ones[:],
                    rhs=terms[:, a:b],
                    start=(mm_state["n"] == 1),
                    stop=(mm_state["n"] == mm_state["total"]),
                )
            del state[t]

        stages = [s0_dma, s1_up, s2_l1, s3_l2, s4_r, s5_wneg, s6_area]
        for t in range(n_tiles + len(stages) - 1):
            for k, fn in enumerate(stages):
                tt = t - k
                if 0 <= tt < n_tiles:
                    fn(tt)

        # psum -> SBUF -> HBM; host does the final 512-wide reduction.
        pout = const_pool.tile([1, PS], f32)
        nc.vector.tensor_copy(pout[:], psum[:])
        nc.sync.dma_start(out=out.ap(), in_=pout[:])

    nc.compile()
    return nc


def _get_prog(ego_vals):
    global _prog, _prog_key
    key = tuple(tuple(r) for r in ego_vals)
    if _prog is None or _prog_key != key:
        _prog = _build_program(ego_vals)
        _prog_key = key
    return _prog


def _ego_aabb(sdc_traj_all, sdc_planning_gt):
    """Per-future ego AABB [F,4] = (xa1, xa2, ya1, ya2), mirroring reference."""
    sdc_traj_all = np.asarray(sdc_traj_all, dtype=np.float32)
    sdc_planning_gt = np.asarray(sdc_planning_gt, dtype=np.float32)
    x = sdc_traj_all[0, :, 0]
    y = sdc_traj_all[0, :, 1]
    theta = sdc_planning_gt[0, :, 2]
    local = np.array(
        [[W / 2, -H / 2], [W / 2, H / 2], [-W / 2, H / 2], [-W / 2, -H / 2]],
        dtype=np.float32,
    )
    c, s = np.cos(theta), np.sin(theta)
    rot = np.stack([np.stack([c, s], -1), np.stack([-s, c], -1)], -2)  # [F,2,2]
    corners = np.einsum("fij,kj->fki", rot, local) + np.stack([x, y], -1)[:, None, :]
    corners = corners.astype(np.float32)
    xa1 = corners[..., 0].max(-1)
    ya1 = corners[..., 1].max(-1)
    xa2 = corners[..., 0].min(-1)
    ya2 = corners[..., 1].min(-1)
    return np.stack([xa1, xa2, ya1, ya2], -1).astype(np.float32)  # [F,4]


def _layout_half(qhalf, name):
    """[F, PADDED, 2, 2] -> {name_f: [P, 4*BPR]} in chunked plane order."""
    # planes [F, P, 4, BPR], plane idx q = corner*2+coord
    a = qhalf.reshape(F, P, BPR, 4).transpose(0, 1, 3, 2)
    outs = {}
    for f in range(F):
        blocks = []
        j = 0
        for w in CHUNKS[f]:
            blocks.append(a[f, :, :, j : j + w].reshape(P, 4 * w))
            j += w
        outs[f"{name}_{f}"] = np.ascontiguousarray(np.concatenate(blocks, axis=1))
    return outs


def _layout_core(m16core, q8core):
    """corners halves ([F, PER_CORE, 2, 2] fp16 / fp8) -> dram arrays."""
    import ml_dtypes

    pad16 = np.full((F, PADDED - PER_CORE, 2, 2), SENTINEL, dtype=np.float16)
    pad8 = np.full(
        (F, PADDED - PER_CORE, 2, 2), SENTINEL, dtype=ml_dtypes.float8_e3m4
    )
    outs = _layout_half(np.concatenate([m16core, pad16], axis=1), "pl16")
    outs.update(_layout_half(np.concatenate([q8core, pad8], axis=1), "pl8"))
    return outs


def kernel(sdc_traj_all, sdc_planning_gt, sdc_planning_gt_mask, future_gt_corners, box_mask):
    import ml_dtypes
    from concourse.bass_utils import run_bass_kernel_spmd

    corners = np.asarray(future_gt_corners, dtype=np.float32)
    mask = np.asarray(box_mask)
    masked = np.where(mask[..., None, None] != 0, corners, np.float32(SENTINEL))
    # corners 0-1 uploaded as fp16 (no device upconvert), corners 2-3 as fp8
    m16 = masked[:, :, 0:2, :].astype(np.float16)
    q8 = masked[:, :, 2:4, :].astype(ml_dtypes.float8_e3m4)

    eg = _ego_aabb(sdc_traj_all, sdc_planning_gt)  # [F,4] = (xa1, xa2, ya1, ya2)
    ego_vals = [[float(eg[f, k]) for k in range(4)] for f in range(F)]

    in_maps = []
    for cidx in range(CORES):
        lo, hi = cidx * PER_CORE, (cidx + 1) * PER_CORE
        in_maps.append(_layout_core(m16[:, lo:hi], q8[:, lo:hi]))

    global _last_in_maps
    _last_in_maps = in_maps
    res = run_bass_kernel_spmd(_get_prog(ego_vals), in_maps, list(range(CORES))).results
    total = 0.0
    for r in res:
        total += float(r["out"].astype(np.float64).sum())
    return np.array([total], dtype=np.float32) * np.float32(WEIGHT)
